# revision 1
# baseline (speedup 1.0000x reference)
"""Trainium2 Bass kernel for CustomBSplineLayer.

Computes out[b,o] = sum_{i,g} spline(x)[b,i,g] * coef[o,i,g] where
spline is an order-3 (cubic) B-spline basis on uniform knots applied to
tanh(x).

Math used here (validated against the reference recursion):
  u = 3.5*tanh(x) + 3.5           in (0, 7)
  basis_g(u) = M4(u - g)          cardinal cubic B-spline, g = 0..7
  M4(s) = (relu(2-|s-2|)^3 - 4*relu(1-|s-2|)^3) / 6
Plane g=7 is identically zero because its support starts at u=7 ==
tanh upper bound, so only 7 of 8 planes contribute (K = 7*1024 = 7168).

Per-core layout (data-parallel over batch, 8 cores x 512 rows):
  - host pre-transposes x so that tiles arrive as [i partitions, b cols]
  - basis planes computed in [i, b] layout feed the PE directly as the
    stationary (lhsT) operand; coef (host-rearranged to [g, i, o]) is the
    moving operand; out accumulates in PSUM as [b, o] across all 56
    k-tiles, then is copied out once.
  - matmul runs in float32r (tf32) which streams at 1 col/cycle for
    free-dim >= 256 (fp32 is 4 cycles/col).
"""

import sys

sys.path.insert(0, "/opt/trn_rl_repo")

import numpy as np
from contextlib import ExitStack

import concourse.bass as bass
import concourse.tile as tile
from concourse import bacc, mybir
from concourse.bass_utils import run_bass_kernel_spmd

F32 = mybir.dt.float32
F32R = mybir.dt.float32r
I32 = mybir.dt.int32
AF = mybir.ActivationFunctionType
OP = mybir.AluOpType

B, I, O = 4096, 1024, 1024
G = 7                    # active basis planes (plane 7 == 0)
NCORES = 8
BC = B // NCORES         # 512 batch rows per core
IT = I // 128            # 8 i-tiles
KT = IT * G              # 56 k-tiles of 128
WID = G * BC             # 3584: wide free-dim (7 planes x 512 b)

C6 = float(6.0 ** (-1.0 / 3.0))          # folds the 1/6 into p
C46 = float((4.0 / 6.0) ** (1.0 / 3.0))  # folds the 4/6 into q
KQ = float(C46 / C6)                     # q = relu(KQ*p - C46)

# mm dtype: F32R (tf32, fast) or F32 (exact, 4x slower PE)
MM_DT = F32R

LAST_RESULT = None  # BassKernelResults of the most recent run (for test.py)

_cache = {}


def _tf32_round(a: np.ndarray) -> np.ndarray:
    """Round fp32 to tf32 (10-bit mantissa), round-to-nearest-even."""
    bits = np.ascontiguousarray(a, dtype=np.float32).view(np.uint32).copy()
    lsb = (bits >> np.uint32(13)) & np.uint32(1)
    bits += np.uint32(0xFFF) + lsb
    bits &= np.uint32(0xFFFFE000)
    return bits.view(np.float32)


def _build_nc(repeats: int = 1):
    nc = bacc.Bacc("TRN2", target_bir_lowering=False, debug=False)
    xT = nc.dram_tensor("xT", [I, BC], F32, kind="ExternalInput").ap()
    coefT = nc.dram_tensor("coefT", [G, I, O], MM_DT, kind="ExternalInput").ap()
    y = nc.dram_tensor("y", [BC, O], F32, kind="ExternalOutput").ap()

    with tile.TileContext(nc) as tc, ExitStack() as ctx:
        xt_pool = ctx.enter_context(tc.tile_pool(name="xt", bufs=2))
        small = ctx.enter_context(tc.tile_pool(name="small", bufs=2))
        wide = ctx.enter_context(tc.tile_pool(name="wide", bufs=2))
        spl_pool = ctx.enter_context(tc.tile_pool(name="spl", bufs=2))
        rhs_pool = ctx.enter_context(tc.tile_pool(name="rhs", bufs=3))
        out_pool = ctx.enter_context(tc.tile_pool(name="ot", bufs=2))
        psum_pool = ctx.enter_context(
            tc.tile_pool(name="psum", bufs=1, space=bass.MemorySpace.PSUM)
        )

        consts = ctx.enter_context(tc.tile_pool(name="consts", bufs=1))
        bias_p = consts.tile([128, 1], F32, tag="bias_p", name="bias_p")
        nc.gpsimd.memset(bias_p[:], 2.0 * C6)
        bias_q = consts.tile([128, 1], F32, tag="bias_q", name="bias_q")
        nc.gpsimd.memset(bias_q[:], -C46)

        # 8 PSUM banks: [m-tile 0..3] x [o-half 0..1], each [128, 512] f32
        psum = [
            [
                psum_pool.tile(
                    [128, 512], F32, tag=f"ps{m}_{h}", name=f"ps{m}_{h}"
                )
                for h in range(2)
            ]
            for m in range(4)
        ]

        def emit_front(rep, it):
            """DMA + tanh + a-planes for i-tile `it` (stage A: ACT+DVE)."""
            xt = xt_pool.tile([128, BC], F32, tag="xt", name=f"xt{rep}_{it}")
            nc.sync.dma_start(xt[:], xT[it * 128 : (it + 1) * 128, :])
            t = small.tile([128, BC], F32, tag="t", name=f"t{rep}_{it}")
            nc.scalar.activation(t[:], xt[:], AF.Tanh)
            # w_g = u-(g+2) = 3.5*t + (1.5-g); one wide sign-bit clear (int
            # AND) turns all 7 planes into a_g = |w_g| at once
            aw = wide.tile([128, WID], F32, tag="a", name=f"aw{rep}_{it}")
            for g in range(G):
                nc.vector.tensor_scalar(
                    aw[:, g * BC : (g + 1) * BC],
                    t[:],
                    3.5,
                    float(1.5 - g),
                    OP.mult,
                    OP.add,
                )
            awi = aw[:].bitcast(I32)
            nc.vector.tensor_scalar(awi, awi, 0x7FFFFFFF, None, OP.bitwise_and)
            return aw

        def emit_mids(rep, it, aw, chunks=1):
            """ACT middle stage: p, q, p^2, q^2 for i-tile `it`."""
            pw = wide.tile([128, WID], F32, tag="p", name=f"pw{rep}_{it}")
            qw = wide.tile([128, WID], F32, tag="q", name=f"qw{rep}_{it}")
            p2 = wide.tile([128, WID], F32, tag="p2", name=f"p2{rep}_{it}")
            q2 = wide.tile([128, WID], F32, tag="q2", name=f"q2{rep}_{it}")
            cw = WID // chunks
            for c in range(chunks):
                s = slice(c * cw, (c + 1) * cw)
                nc.scalar.activation(
                    pw[:, s], aw[:, s], AF.Relu, bias=bias_p[:], scale=-C6
                )
                nc.scalar.activation(
                    qw[:, s], pw[:, s], AF.Relu, bias=bias_q[:], scale=KQ
                )
                nc.scalar.activation(p2[:, s], pw[:, s], AF.Square)
                nc.scalar.activation(q2[:, s], qw[:, s], AF.Square)
            return pw, qw, p2, q2

        def emit_cubes(rep, it, mids, chunks=1):
            """DVE cube stage: p2 *= p, q2 *= q (in place), spl = p3 - q3."""
            pw, qw, p2, q2 = mids
            spl = spl_pool.tile([128, WID], MM_DT, tag="spl", name=f"spl{rep}_{it}")
            cw = WID // chunks
            for c in range(chunks):
                s = slice(c * cw, (c + 1) * cw)
                nc.vector.tensor_tensor(p2[:, s], p2[:, s], pw[:, s], OP.mult)
                nc.vector.tensor_tensor(q2[:, s], q2[:, s], qw[:, s], OP.mult)
                # subtract writes an fp32r-typed tile: the DVE rounds to
                # tf32 on write, as the fp32r matmul requires of producers
                nc.vector.tensor_tensor(spl[:, s], p2[:, s], q2[:, s], OP.subtract)
            return spl

        def emit_matmuls(rep, it, spl, kt):
            for g in range(G):
                rhs = rhs_pool.tile(
                    [128, O], MM_DT, tag="rhs", name=f"rhs{rep}_{it}_{g}"
                )
                nc.sync.dma_start(rhs[:], coefT[g, it * 128 : (it + 1) * 128, :])
                first = kt == 0
                last = kt == KT - 1
                for m in range(4):
                    lhsT = spl[:, g * BC + m * 128 : g * BC + (m + 1) * 128]
                    for h in range(2):
                        nc.tensor.matmul(
                            psum[m][h][:],
                            lhsT,
                            rhs[:, h * 512 : (h + 1) * 512],
                            start=first,
                            stop=last,
                        )
                kt += 1
            return kt

        for _rep in range(repeats):
            # software-pipelined emission: within each block the DVE first
            # produces a(it), then (while ACT runs mids(it)) finishes the
            # cubes of it-1, whose matmuls follow immediately.
            kt = 0
            # i-tile 0 runs per-plane (chunks=G) so its first matmuls can
            # start ~20us earlier (deps are tracked per slice); later tiles
            # use full-wide ops.
            ch0 = G if _rep == 0 else 1
            aw = emit_front(_rep, 0)
            mids = emit_mids(_rep, 0, aw, chunks=ch0)
            for it in range(1, IT):
                aw = emit_front(_rep, it)
                prev_mids = mids
                mids = emit_mids(_rep, it, aw)
                spl = emit_cubes(_rep, it - 1, prev_mids, chunks=ch0 if it == 1 else 1)
                kt = emit_matmuls(_rep, it - 1, spl, kt)
            spl = emit_cubes(_rep, IT - 1, mids)
            kt = emit_matmuls(_rep, IT - 1, spl, kt)

            for m in range(4):
                ot = out_pool.tile([128, O], F32, tag="ot", name=f"ot{_rep}_{m}")
                for h in range(2):
                    nc.scalar.copy(ot[:, h * 512 : (h + 1) * 512], psum[m][h][:])
                nc.sync.dma_start(y[m * 128 : (m + 1) * 128, :], ot[:])

    nc.compile()
    return nc


def kernel(x: np.ndarray, coef: np.ndarray) -> np.ndarray:
    global LAST_RESULT
    x = np.asarray(x, dtype=np.float32)
    coef = np.asarray(coef, dtype=np.float32)
    assert x.shape == (B, I) and coef.shape == (O, I, 8)

    if "nc" not in _cache:
        _cache["nc"] = _build_nc()
    nc = _cache["nc"]

    xT = np.ascontiguousarray(x.T)  # [I, B]
    coefT = np.ascontiguousarray(coef.transpose(2, 1, 0)[:G])  # [7, I, O]
    if MM_DT == F32R:
        coefT = _tf32_round(coefT)
    in_maps = [
        {
            "xT": np.ascontiguousarray(xT[:, c * BC : (c + 1) * BC]),
            "coefT": coefT,
        }
        for c in range(NCORES)
    ]
    res = run_bass_kernel_spmd(nc, in_maps, list(range(NCORES)))
    LAST_RESULT = res
    out = np.concatenate([res.results[c]["y"] for c in range(NCORES)], axis=0)
    return np.ascontiguousarray(out.astype(np.float32))


if __name__ == "__main__":
    rng = np.random.default_rng(0)
    x = rng.standard_normal((B, I), dtype=np.float32)
    coef = rng.standard_normal((O, I, 8), dtype=np.float32) * 0.1
    out = kernel(x, coef)
    print("out", out.shape, out.dtype, float(np.abs(out).max()))



# revision 22
# speedup vs baseline: 1.0857x; 1.0857x over previous
"""Trainium2 Bass kernel for CustomBSplineLayer (v2: fp16 pipeline).

out[b,o] = sum_{i,g} spline(x)[b,i,g] * coef[o,i,g], spline = cubic
B-spline basis on uniform knots applied to u = 3.5*tanh(x) + 3.5 in
(0,7); basis_g(u) = M4(u - g), plane 7 is identically zero -> 7 planes.

M4 via two relu-cube branches with folded constants (numerically stable):
  a   = |u - (g+2)| = abs_max(u - (g+2), 0)     (one DVE ts op per plane)
  p   = relu(-C6*a + 2*C6)                      (C6 = 6^(-1/3))
  q   = relu(KQ*p - C46)                        (folds the 4/6)
  s_g = p^3 - q^3

Engine split per i-tile (7 planes, [128, 3584] wide tiles, cols g*512+b):
  ACT : tanh, p = Relu(aff(a)), p2 = Square(aff(a)), q = Relu(aff(p))
        (Square of the affine is valid: p3 = p2*p and p = relu kills z<0)
  DVE : u (ts 4x), a_g (ts abs_max 4x), p3 = p2*p, q3 = q2*q, s = p3-q3
  Pool: q2 = q*q
  PE  : fp16 matmul, 56 k-tiles x 8 psum banks, 512-col movings
  out : DMA straight from PSUM to DRAM (no drain copies)

Data-parallel over batch: 8 cores x 512 rows; coef packed fp16 [7, I, O]
host-side. DMA per rep: 14.7MB coef + 2MB x + 2MB out << PE time.
"""

import sys

sys.path.insert(0, "/opt/trn_rl_repo")

import numpy as np
from contextlib import ExitStack

import concourse.bass as bass
import concourse.tile as tile
from concourse import bacc, mybir
from concourse.bass_utils import run_bass_kernel_spmd

F32 = mybir.dt.float32
FP16 = mybir.dt.float16
AF = mybir.ActivationFunctionType
OP = mybir.AluOpType

B, I, O = 4096, 1024, 1024
G = 7                    # active basis planes (plane 7 == 0)
NCORES = 8
BC = B // NCORES         # 512 batch rows per core
IT = I // 128            # 8 i-tiles
WID = G * 512            # 3584 cols per [128, WID] wide tile

C6 = float(6.0 ** (-1.0 / 3.0))
C46 = float((4.0 / 6.0) ** (1.0 / 3.0))
KQ = float(C46 / C6)

MM_DT = FP16

LAST_RESULT = None

_cache = {}


def _build_nc(repeats: int = 1):
    nc = bacc.Bacc("TRN2", target_bir_lowering=False, debug=False)
    xT = nc.dram_tensor("xT", [I, BC], F32, kind="ExternalInput").ap()
    coefT = nc.dram_tensor("coefT", [G, I, O], FP16, kind="ExternalInput").ap()
    y = nc.dram_tensor("y", [BC, O], F32, kind="ExternalOutput").ap()

    with tile.TileContext(nc) as tc, ExitStack() as ctx:
        xt_pool = ctx.enter_context(tc.tile_pool(name="xt", bufs=2))
        tu_pool = ctx.enter_context(tc.tile_pool(name="tu", bufs=2))
        wideA = ctx.enter_context(tc.tile_pool(name="wa", bufs=2))
        wideB = ctx.enter_context(tc.tile_pool(name="wb", bufs=2))
        spl_pool = ctx.enter_context(tc.tile_pool(name="spl", bufs=2))
        rhs_pool = ctx.enter_context(tc.tile_pool(name="rhs", bufs=3))
        out_pool = ctx.enter_context(tc.tile_pool(name="ot", bufs=2))
        psum_pool = ctx.enter_context(
            tc.tile_pool(name="psum", bufs=1, space=bass.MemorySpace.PSUM)
        )

        consts = ctx.enter_context(tc.tile_pool(name="consts", bufs=1))
        bias_p = consts.tile([128, 1], F32, tag="bias_p", name="bias_p")
        nc.gpsimd.memset(bias_p[:], 2.0 * C6)
        bias_q = consts.tile([128, 1], F32, tag="bias_q", name="bias_q")
        nc.gpsimd.memset(bias_q[:], -C46)
        dmy_l = consts.tile([128, 128], FP16, tag="dmy_l", name="dmy_l")
        nc.gpsimd.memset(dmy_l[:], 0.0)
        dmy_r = consts.tile([128, 512], FP16, tag="dmy_r", name="dmy_r")
        nc.gpsimd.memset(dmy_r[:], 0.0)

        # 8 PSUM banks: [m-tile 0..3] x [o-half 0..1], each [128, 512] f32
        psum = [
            [
                psum_pool.tile([128, 512], F32, tag=f"ps{m}_{h}", name=f"ps{m}_{h}")
                for h in range(2)
            ]
            for m in range(4)
        ]

        def emit_spline(rep, it, chunks):
            """Elementwise pipeline for i-tile -> s tile [128, WID] fp16."""
            xt = xt_pool.tile([128, 512], F32, tag="xt", name=f"xt{rep}_{it}")
            nc.sync.dma_start(xt[:], xT[it * 128:(it + 1) * 128, :])
            t = tu_pool.tile([128, 512], FP16, tag="t", name=f"t{rep}_{it}")
            nc.scalar.activation(t[:], xt[:], AF.Tanh)
            u = tu_pool.tile([128, 512], FP16, tag="u", name=f"u{rep}_{it}")
            nc.vector.tensor_scalar(u[:], t[:], 3.5, 3.5, OP.mult, OP.add)

            a = wideA.tile([128, WID], FP16, tag="a", name=f"a{rep}_{it}")
            for g in range(G):
                nc.vector.tensor_scalar(
                    a[:, g * 512:(g + 1) * 512],
                    u[:], float(-(g + 2)), None, OP.add,
                )
                if chunks > 1:
                    ai = a[:, g * 512:(g + 1) * 512].bitcast(mybir.dt.int16)
                    nc.vector.tensor_scalar(ai, ai, 0x7FFF, None, OP.bitwise_and)
            if chunks == 1:
                ai = a[:].bitcast(mybir.dt.int16)
                nc.vector.tensor_scalar(ai, ai, 0x7FFF, None, OP.bitwise_and)
            p = wideA.tile([128, WID], FP16, tag="p", name=f"p{rep}_{it}")
            p2 = wideB.tile([128, WID], FP16, tag="p2", name=f"p2{rep}_{it}")
            q = wideA.tile([128, WID], FP16, tag="q", name=f"q{rep}_{it}")
            q2 = wideB.tile([128, WID], FP16, tag="q2", name=f"q2{rep}_{it}")
            s = spl_pool.tile([128, WID], MM_DT, tag="s", name=f"s{rep}_{it}")
            cw = WID // chunks
            for c in range(chunks):
                sl = slice(c * cw, (c + 1) * cw)
                nc.scalar.activation(
                    p[:, sl], a[:, sl], AF.Relu, bias=bias_p[:], scale=-C6
                )
                nc.scalar.activation(
                    q[:, sl], p[:, sl], AF.Relu, bias=bias_q[:], scale=KQ
                )
                if chunks > 1:
                    # fill path: keep ACT lean, fan p2/q2/s out to Pool
                    nc.gpsimd.tensor_tensor(p2[:, sl], p[:, sl], p[:, sl], OP.mult)
                    nc.gpsimd.tensor_tensor(q2[:, sl], q[:, sl], q[:, sl], OP.mult)
                    nc.vector.tensor_tensor(p2[:, sl], p2[:, sl], p[:, sl], OP.mult)
                    nc.vector.tensor_tensor(q2[:, sl], q2[:, sl], q[:, sl], OP.mult)
                    nc.gpsimd.tensor_tensor(s[:, sl], p2[:, sl], q2[:, sl], OP.subtract)
                else:
                    # steady path: p2 = Square(aff(a)) fused on ACT (z^2;
                    # valid since p3 = p2*p and p = relu kills z < 0)
                    nc.scalar.activation(
                        p2[:, sl], a[:, sl], AF.Square, bias=bias_p[:], scale=-C6
                    )
                    nc.gpsimd.tensor_tensor(q2[:, sl], q[:, sl], q[:, sl], OP.mult)
                    nc.vector.tensor_tensor(p2[:, sl], p2[:, sl], p[:, sl], OP.mult)
                    nc.vector.tensor_tensor(q2[:, sl], q2[:, sl], q[:, sl], OP.mult)
                    nc.vector.tensor_tensor(s[:, sl], p2[:, sl], q2[:, sl], OP.subtract)
            return s

        def emit_matmuls(rep, it, s, kt):
            for g in range(G):
                rhs = rhs_pool.tile(
                    [128, O], MM_DT, tag="rhs", name=f"rhs{rep}_{it}_{g}"
                )
                nc.sync.dma_start(rhs[:], coefT[g, it * 128:(it + 1) * 128, :])
                first = kt == 0
                last = kt == G * IT - 1
                for m in range(4):
                    lhsT = s[:, g * 512 + m * 128:g * 512 + (m + 1) * 128]
                    for h in range(2):
                        nc.tensor.matmul(
                            psum[m][h][:],
                            lhsT,
                            rhs[:, h * 512:(h + 1) * 512],
                            start=first,
                            stop=last,
                        )
                kt += 1
            return kt

        def emit_last_itile_bank_major(rep, it, s):
            """Bank-major matmuls for the final i-tile: each PSUM bank gets
            its stop early, so copy-out + DMA overlap remaining matmuls."""
            rhs = []
            for g in range(G):
                r = rhs_pool.tile(
                    [128, O], MM_DT, tag=f"rhs7_{g}", name=f"rhs7_{rep}_{g}"
                )
                nc.sync.dma_start(r[:], coefT[g, it * 128:(it + 1) * 128, :])
                rhs.append(r)
            for m in range(4):
                for h in range(2):
                    for g in range(G):
                        lhsT = s[:, g * 512 + m * 128:g * 512 + (m + 1) * 128]
                        nc.tensor.matmul(
                            psum[m][h][:],
                            lhsT,
                            rhs[g][:, h * 512:(h + 1) * 512],
                            start=False,
                            stop=(g == G - 1),
                        )
                    ot = out_pool.tile(
                        [128, 512], F32, tag=f"ot{h}", name=f"ot{rep}_{m}_{h}"
                    )
                    nc.vector.tensor_scalar(ot[:], psum[m][h][:], 0.0, None, OP.add)
                    nc.sync.dma_start(
                        y[m * 128:(m + 1) * 128, h * 512:(h + 1) * 512], ot[:]
                    )

        for rep in range(repeats):
            kt = 0
            if rep == 0:
                # warm the PE p-state during the fill with throwaway matmuls
                for j in range(17):
                    nc.tensor.matmul(
                        psum[0][0][:], dmy_l[:], dmy_r[:], start=True, stop=True
                    )
            chunk_sched = {0: G, 1: 4, 2: 2} if rep == 0 else {}
            s = emit_spline(rep, 0, chunks=chunk_sched.get(0, 1))
            for it in range(1, IT):
                s_next = emit_spline(rep, it, chunks=chunk_sched.get(it, 1))
                kt = emit_matmuls(rep, it - 1, s, kt)
                s = s_next
            emit_last_itile_bank_major(rep, IT - 1, s)

    nc.compile()
    return nc


def make_in_maps(x: np.ndarray, coef: np.ndarray):
    """Pack full inputs into per-core input maps for the bass program."""
    x = np.asarray(x, dtype=np.float32)
    coef = np.asarray(coef, dtype=np.float32)
    xT = np.ascontiguousarray(x.T)  # [I, B]
    cT = np.ascontiguousarray(
        coef[:, :, :G].transpose(2, 1, 0)
    ).astype(np.float16)  # [7, I, O]
    return [
        {
            "xT": np.ascontiguousarray(xT[:, c * BC:(c + 1) * BC]),
            "coefT": cT,
        }
        for c in range(NCORES)
    ]


def kernel(x: np.ndarray, coef: np.ndarray) -> np.ndarray:
    global LAST_RESULT
    x = np.asarray(x, dtype=np.float32)
    coef = np.asarray(coef, dtype=np.float32)
    assert x.shape == (B, I) and coef.shape == (O, I, 8)

    if "nc" not in _cache:
        _cache["nc"] = _build_nc()
    nc = _cache["nc"]

    in_maps = make_in_maps(x, coef)
    res = run_bass_kernel_spmd(nc, in_maps, list(range(NCORES)))
    LAST_RESULT = res
    out = np.concatenate([res.results[c]["y"] for c in range(NCORES)], axis=0)
    return np.ascontiguousarray(out.astype(np.float32))


if __name__ == "__main__":
    rng = np.random.default_rng(0)
    x = rng.standard_normal((B, I), dtype=np.float32)
    coef = rng.standard_normal((O, I, 8), dtype=np.float32) * 0.1
    out = kernel(x, coef)
    print("out", out.shape, out.dtype, float(np.abs(out).max()))
